# revision 23
# baseline (speedup 1.0000x reference)
"""MultiHeadAttention Trainium2 kernel.

Full inputs: x [4, 2048, 768] f32, W_qkv [2304, 768], W_proj [768, 768],
b_proj [768]. Output [4, 2048, 768] f32.

Sharding: 8 cores = 4 batches x 2 head-groups (6 heads each).
Per-core inputs (host-prepared, transposed on host):
  xT  [768, 2048]  = x[b].T
  wT  [768, 1152]  = concat(Wq_g, Wk_g, Wv_g).T   (g = head group rows)
  wpT [384, 768]   = W_proj[:, g-cols].T
Per-core output: outp [2048, 768] = partial projection output for batch b.
Host: out[b] = outp[2b] + outp[2b+1] + b_proj.

On-device (per core) -- ACT(exp)-bound design, ~25.2M exps per core:
  * QK energies per head-pair m via ROW-TILED K=64 matmuls: head 2m lives in
    SBUF partitions 0:64, head 2m+1 in 64:128 of qT/kT2 slot m; the two
    K=64 matmuls (base partitions 0 / 64) run concurrently on disjoint
    row-groups of the PE array -> 2 heads per 512-col stream.
  * One exp ACT call per kk covers both heads' energy banks (N=1024), with
    scale=1/8 folded in, no max subtraction (energies O(10)).
  * AV: av[65,q] += v_aug.T @ e; row 64 (ones col of v_aug) is the softmax
    denominator.
  * Loop order n-OUTER (q-block), head-pair inner. All non-attention work is
    threaded INTO the ACT-bound kk loops so no engine FIFO ever blocks the
    QK->exp stream:
      - pre-phase projections run c-outer over 8 concurrent PSUM
        accumulators so they pipeline with the input DMA;
      - remaining q/k/v projections are "filler" generators pumped 2 matmuls
        at a time between QK and AV;
      - each pair's softmax normalize chain (batched Newton reciprocal on
        standard DVE ops -> gpsimd partition-broadcast -> DVE mul) is sliced
        into single-op slots executed during the NEXT pair's kk loop (the
        DVE InstReciprocal is avoided mid-stream because the scheduler's
        cost model under-prices its iterative divide ~6x and then hoists
        dependent phase-3 matmuls into PE-FIFO positions that stall);
      - phase 3 (output proj) for q-block n is pumped during the second half
        of the pair after its last normalize; only q-block 3's runs in the
        tail, with junk matmuls keeping the PE clock-gate open.
  * PSUM: eps [128,2,512]x2 (energies ping/pong), av x2 (AV accumulators),
    ms x2 (projection / phase-3 tiles) = 8 banks.
"""

import ml_dtypes
import numpy as np

import concourse.bass as bass
import concourse.tile as tile
from concourse import bacc, mybir
from concourse.bass_utils import run_bass_kernel_spmd

EMB = 768
N = 2048
B = 4
D = 64
HL = 6            # heads per core
HD = HL * D       # 384 local head-dim columns
NCORES = 8
SCALE = D ** -0.5

F32 = mybir.dt.float32
BF16 = mybir.dt.bfloat16

EC = EMB // 128   # 6 emb chunks
MC = HD // 128    # 3 local head-dim chunks (= head pairs)
NQ = N // 512     # 4 query blocks of 512
NK = N // 128     # 16 key/seq chunks of 128

EXP = mybir.ActivationFunctionType.Exp


def _emit(tc):
    from contextlib import ExitStack

    nc = tc.nc
    xT = nc.dram_tensor("xT", [EMB, N], BF16, kind="ExternalInput").ap()
    wT = nc.dram_tensor("wT", [EMB, 3 * HD], BF16, kind="ExternalInput").ap()
    wpT = nc.dram_tensor("wpT", [HD, EMB], BF16, kind="ExternalInput").ap()
    outp = nc.dram_tensor("outp", [N, EMB], F32, kind="ExternalOutput").ap()

    xTr = xT.rearrange("(c p) s -> p c s", p=128)
    wTr = wT.rearrange("(c p) s -> p c s", p=128)
    wpTr = wpT.rearrange("(m p) e -> p m e", p=128)
    outr = outp.rearrange("(s p) e -> p s e", p=128)

    with ExitStack() as persist:
        ppool = persist.enter_context(tc.tile_pool(name="persist", bufs=1))
        warm_sb = ppool.tile([128, 640], BF16)
        nc.vector.memset(warm_sb[:], 1.0)
        wp_sb = ppool.tile([128, MC, EMB], BF16)
        # per-slice tiles: slice-level dependency tracking is coarse within a
        # tile, so sharing one big tile creates false cross-engine deps that
        # stall the PE FIFO (e.g. phase-3 reads of q-block n-1 serialized
        # behind normalize-mul writes of q-block n)
        qT_sb = [ppool.tile([128, N], BF16, name=f"qT{i}") for i in range(MC)]
        kT2_sb = [ppool.tile([128, N], BF16, name=f"kT{i}") for i in range(MC)]
        v_sb = [ppool.tile([128, HL * (D + 1) + D], BF16, name=f"v{i}")
                for i in range(NK)]
        for vt in v_sb:
            nc.vector.memset(vt[:], 1.0)
        attT_sb = [ppool.tile([128, MC, 512], BF16, name=f"attT{i}")
                   for i in range(NQ)]
        x_sb = [ppool.tile([128, N], BF16, name=f"x{i}") for i in range(EC)]
        # w is written once by a single DMA then read-only, so one tile (and
        # one Sync-engine descriptor issue instead of six) is safe and shaves
        # the serialized-issue latency off the DMA-bound start
        w_sb = ppool.tile([128, EC, 3 * HD], BF16)

        psum_pool = persist.enter_context(
            tc.tile_pool(name="psum", bufs=1, space="PSUM"))
        esb_pool = persist.enter_context(tc.tile_pool(name="esb", bufs=4))
        sm_pool = persist.enter_context(tc.tile_pool(name="sm", bufs=4))
        osb_pool = persist.enter_context(tc.tile_pool(name="osb", bufs=3))

        def ms_tile(name):
            return psum_pool.tile([128, 512], F32, tag="ms", bufs=2, name=name)

        def eps_tile(name):
            return psum_pool.tile([128, 2, 512], F32, tag="eps", bufs=2,
                                  name=name)

        def av_tile(name):
            return psum_pool.tile([128, 512], F32, tag="av", bufs=2, name=name)

        # PE warmup junk so the HAM clock-gate opens during the input-DMA wait
        warm_ps = ms_tile("warm_ps")
        for wi in range(10):
            nc.tensor.matmul(warm_ps[:], warm_sb[:, 0:128], warm_sb[:, 128:640],
                             start=(wi == 0), stop=(wi == 9))

        nc.sync.dma_start(w_sb[:], wTr)
        for c in range(EC):
            nc.sync.dma_start(x_sb[c][:], xTr[:, c, :])
        nc.sync.dma_start(wp_sb[:], wpTr)

        # ---- projection chains (q/k/v from x), c-steppable ----
        class Chain:
            """One psum accumulation chain: 6 matmuls (one per emb chunk)
            then a cast to the bf16 destination."""

            def __init__(self, kind, a, n, view):
                self.kind, self.a, self.n, self.view = kind, a, n, view

            def mm(self, c):
                v = self.view
                if self.kind == "v":
                    nc.tensor.matmul(
                        v[:], x_sb[c][:, self.a * 128:(self.a + 1) * 128],
                        w_sb[:, c, 2 * HD:3 * HD],
                        start=(c == 0), stop=(c == EC - 1))
                else:
                    lo = (0 if self.kind == "q" else HD) + self.a * 128
                    ns = slice(self.n * 512, (self.n + 1) * 512)
                    nc.tensor.matmul(v[:], w_sb[:, c, lo:lo + 128],
                                     x_sb[c][:, ns],
                                     start=(c == 0), stop=(c == EC - 1))

            def cast(self):
                v = self.view
                if self.kind == "v":
                    nc.vector.tensor_copy(
                        v_sb[self.a][:, 0:HL * (D + 1)].rearrange(
                            "p (h c) -> p h c", c=D + 1)[:, :, 0:D],
                        v[:].rearrange("p (h d) -> p h d", h=HL))
                else:
                    ns = slice(self.n * 512, (self.n + 1) * 512)
                    dst = qT_sb if self.kind == "q" else kT2_sb
                    nc.vector.tensor_copy(dst[self.a][:, ns], v[:])

        # pre-phase group A: 8 concurrent chains (all 8 psum banks), c-outer
        # so matmuls stream in as each x/w chunk lands.
        e1 = eps_tile("pre_e1")
        e2 = eps_tile("pre_e2")
        ga_views = [e1[:, 0, :], e1[:, 1, :], e2[:, 0, :], e2[:, 1, :],
                    ms_tile("pre_ms0")[:, :], ms_tile("pre_ms1")[:, :],
                    av_tile("pre_av0")[:, :], av_tile("pre_av1")[:, :]]
        groupA = [Chain("k", 0, 0, ga_views[0]), Chain("q", 0, 0, ga_views[4]),
                  Chain("k", 0, 1, ga_views[1]), Chain("k", 0, 2, ga_views[2]),
                  Chain("k", 0, 3, ga_views[3]),
                  Chain("v", 0, 0, ga_views[5][:, 0:HD]),
                  Chain("v", 1, 0, ga_views[6][:, 0:HD]),
                  Chain("v", 2, 0, ga_views[7][:, 0:HD])]
        for c in range(EC):
            for ch in groupA:
                ch.mm(c)
        for ch in groupA:
            ch.cast()

        # ---- filler generators: proj chains on ms tiles, 2 matmuls/quantum
        def proj_gen(kind, a, n):
            view = ms_tile(f"f{kind}{a}_{n}")[:, :]
            if kind == "v":
                view = view[:, 0:HD]
            ch = Chain(kind, a, n, view)
            for c in range(EC):
                ch.mm(c)
                if c % 2 == 1:
                    yield
            ch.cast()
            yield

        filler = []

        def pump(k=1):
            done = 0
            while filler and done < k:
                try:
                    next(filler[0])
                    done += 1
                except StopIteration:
                    filler.pop(0)

        # fillers, in need-order (each entry needed before the NEXT pair):
        fill_plan = {
            (0, 0): [("v", s, 0) for s in range(3, NK)]
                    + [("k", 1, 0), ("q", 1, 0), ("k", 1, 1)],
            (0, 1): [("k", 1, 2), ("k", 1, 3)]
                    + [("k", 2, nn) for nn in range(NQ)] + [("q", 2, 0)],
            (0, 2): [("q", mm, 1) for mm in range(MC)],
            (1, 0): [("q", mm, 2) for mm in range(MC)],
            (1, 1): [("q", mm, 3) for mm in range(MC)],
        }

        # ---- normalize-chain slot ops (executed during the next pair) ----
        # 1/l via Newton iteration on standard DVE ops instead of the
        # iterative-divide InstReciprocal: the scheduler's cost model prices
        # reciprocal at 1 elem/cycle while the hardware takes ~6.2, so any
        # work it schedules after a reciprocal-fed dependency stalls the PE
        # FIFO ~3us per occurrence. Newton (bitwise seed + 2 iterations,
        # ~7e-6 max rel err) is priced correctly and batches both heads'
        # denominators in one [65,512] tile (rows 0 and 64; middle junk).
        I32 = mybir.dt.int32
        MAGIC = 0x7EF311C4
        ALU = mybir.AluOpType

        def normalize_slots(m, n, avstA, avstB):
            out = []
            nt = [sm_pool.tile([65, 512], F32, tag="nwt", bufs=14,
                               name=f"nw{i}_{n}_{m}") for i in range(7)]
            den, y0, t1, u1, y1, t2v, y2v = nt
            recB = sm_pool.tile([1, 512], F32, tag="rec", bufs=6,
                                name=f"recB_{n}_{m}")
            rbA = sm_pool.tile([D, 512], F32, tag="rb", bufs=6,
                               name=f"rbA_{n}_{m}")
            rbB = sm_pool.tile([D, 512], F32, tag="rb", bufs=6,
                               name=f"rbB_{n}_{m}")
            out.append(lambda: nc.vector.memset(den[:], 1.0))
            out.append(lambda: nc.vector.tensor_copy(den[0:1, :],
                                                     avstA[D:D + 1, :]))
            out.append(lambda: nc.vector.tensor_copy(den[64:65, :],
                                                     avstB[D:D + 1, :]))
            out.append(lambda: nc.vector.tensor_scalar(
                u1[:].bitcast(I32), den[:].bitcast(I32), -1, None,
                ALU.bitwise_xor))
            out.append(lambda: nc.vector.tensor_scalar(
                y0[:].bitcast(I32), u1[:].bitcast(I32), MAGIC, None,
                ALU.add))
            for tt, uu, yin, yout in ((t1, u1, y0, y1), (t2v, u1, y1, y2v)):
                out.append(lambda tt=tt, yin=yin: nc.vector.tensor_mul(
                    tt[:], den[:], yin[:]))
                out.append(lambda tt=tt, uu=uu: nc.vector.tensor_scalar(
                    uu[:], tt[:], -1.0, 2.0, ALU.mult, ALU.add))
                out.append(lambda yin=yin, uu=uu, yout=yout:
                           nc.vector.tensor_mul(yout[:], yin[:], uu[:]))
            out.append(lambda: nc.vector.tensor_copy(recB[:], y2v[64:65, :]))
            out.append(lambda: nc.gpsimd.partition_broadcast(rbA[:],
                                                             y2v[0:1, :]))
            out.append(lambda: nc.gpsimd.partition_broadcast(rbB[:],
                                                             recB[:]))
            out.append(lambda: nc.vector.tensor_mul(
                attT_sb[n][0:64, m, :], avstA[0:D, :], rbA[:]))
            out.append(lambda: nc.vector.tensor_mul(
                attT_sb[n][64:128, m, :], avstB[0:D, :], rbB[:]))
            return out

        # ---- phase-3 generator for q-block n (pumped as quanta) ----
        def phase3_gen(n):
            for s in range(4 * n, 4 * n + 4):
                o_sb = osb_pool.tile([128, EMB], F32, tag="osb",
                                     name=f"osb_{s}")
                prs = []
                sc = (s - 4 * n) * 128
                for half in range(2):
                    pr = ms_tile(f"pr_{s}_{half}")[:, 0:HD]
                    for mc in range(MC):
                        nc.tensor.matmul(
                            pr[:], attT_sb[n][:, mc, sc:sc + 128],
                            wp_sb[:, mc, half * HD:(half + 1) * HD],
                            start=(mc == 0), stop=(mc == MC - 1))
                    prs.append(pr)
                    yield
                for half in range(2):
                    if n == NQ - 1:
                        # scalar engine is idle during the tail
                        nc.scalar.copy(o_sb[:, half * HD:(half + 1) * HD],
                                       prs[half][:])
                    else:
                        nc.vector.tensor_copy(
                            o_sb[:, half * HD:(half + 1) * HD], prs[half][:])
                nc.sync.dma_start(outr[:, s, :], o_sb[:])
                yield

        # ---- fused attention, n-outer ----
        # Software pipelining: each pair's AV(kk) is emitted after QK(kk+1),
        # and the pair's drain (avst copies + normalize slots) after the NEXT
        # pair's QK(0), so independent QK work always sits ahead of
        # esb/psum-gated work in the PE FIFO.
        slots = []          # cross-engine single-op thunks (normalize)
        ph3 = None          # pending phase-3 generator
        pending_av = None   # AV emission delayed one kk behind its QK
        pending_drain = None
        for n in range(NQ):
            ns = slice(n * 512, (n + 1) * 512)
            for m in range(MC):
                # this pair reads projections queued in the previous pair's
                # plan -- force-drain any spillover before touching them
                while filler:
                    pump(1)
                for kind, pa, pn in fill_plan.get((n, m), []):
                    filler.append(proj_gen(kind, pa, pn))
                hA, hB = 2 * m, 2 * m + 1
                avA = avB = None
                for kk in range(NK):
                    kkr = slice(kk * 128, (kk + 1) * 128)
                    eps = eps_tile(f"eps_{n}_{m}_{kk}")
                    nc.tensor.matmul(eps[:, 0, :], kT2_sb[m][0:64, kkr],
                                     qT_sb[m][0:64, ns], start=True, stop=True)
                    nc.tensor.matmul(eps[:, 1, :], kT2_sb[m][64:128, kkr],
                                     qT_sb[m][64:128, ns], start=True,
                                     stop=True)
                    if pending_av is not None:
                        pending_av()
                        pending_av = None
                    if pending_drain is not None:
                        pending_drain()
                        pending_drain = None
                    if avA is None:
                        avA = av_tile(f"avA_{n}_{m}")
                        avB = av_tile(f"avB_{n}_{m}")
                    pump(2)
                    esb = esb_pool.tile([128, 2, 512], BF16, tag="esb",
                                        name=f"esb_{n}_{m}_{kk}")
                    nc.scalar.activation(esb[:], eps[:], EXP, scale=SCALE)

                    def mk_av(avA=avA, avB=avB, esb=esb, kk=kk, hA=hA, hB=hB):
                        nc.tensor.matmul(
                            avA[:],
                            v_sb[kk][:, hA * (D + 1):hA * (D + 1) + 128],
                            esb[:, 0, :], start=(kk == 0),
                            stop=(kk == NK - 1))
                        nc.tensor.matmul(
                            avB[:],
                            v_sb[kk][:, hB * (D + 1):hB * (D + 1) + 128],
                            esb[:, 1, :], start=(kk == 0),
                            stop=(kk == NK - 1))
                    pending_av = mk_av
                    pump(2)
                    if slots:
                        slots.pop(0)()
                    elif ph3 is not None and kk >= 8:
                        for _ in range(2):
                            next(ph3, None)

                last = (n == NQ - 1) and (m == MC - 1)
                avstA = sm_pool.tile([D + 1, 512], F32, tag="avst", bufs=6,
                                     name=f"avst_{n}_{m}_0")
                avstB = sm_pool.tile([D + 1, 512], F32, tag="avst", bufs=6,
                                     name=f"avst_{n}_{m}_1")
                if not last:
                    def mk_drain(m=m, n=n, avA=avA, avB=avB, avstA=avstA,
                                 avstB=avstB):
                        def drain():
                            nonlocal ph3
                            nc.vector.tensor_copy(avstA[:], avA[0:D + 1, :])
                            nc.vector.tensor_copy(avstB[:], avB[0:D + 1, :])
                            slots.extend(normalize_slots(m, n, avstA, avstB))
                            if m == 0 and n >= 1:
                                if ph3 is not None:
                                    for _ in ph3:
                                        pass
                                ph3 = phase3_gen(n - 1)
                        return drain
                    pending_drain = mk_drain()
                else:
                    # ---- tail: shortest-chain finish for the last pair ----
                    pending_av()
                    pending_av = None
                    nc.scalar.copy(avstA[:], avA[0:D + 1, :])
                    nc.scalar.copy(avstB[:], avB[0:D + 1, :])
                    recA = sm_pool.tile([1, 512], F32, tag="rec", bufs=6,
                                        name="recAl")
                    recB = sm_pool.tile([1, 512], F32, tag="rec", bufs=6,
                                        name="recBl")
                    nc.vector.reciprocal(recA[:], avstA[D:D + 1, :])
                    nc.vector.reciprocal(recB[:], avstB[D:D + 1, :])
                    # junk matmuls keep the PE clock-gate open through the
                    # normalize chain so phase 3 runs at full clock
                    warm2 = ms_tile("warm2")
                    for wi in range(44):
                        nc.tensor.matmul(warm2[:], warm_sb[:, 0:128],
                                         warm_sb[:, 128:640],
                                         start=(wi == 0), stop=(wi == 43))
                    for half, avst, r in ((0, avstA, recA[:]),
                                          (1, avstB, recB[:])):
                        rb = sm_pool.tile([D, 512], F32, tag="rb", bufs=6,
                                          name=f"rb_l_{half}")
                        nc.gpsimd.partition_broadcast(rb[:], r)
                        p0 = half * 64
                        nc.vector.tensor_mul(attT_sb[n][p0:p0 + 64, m, :],
                                             avst[0:D, :], rb[:])

            if n == NQ - 1:
                if ph3 is not None:
                    for _ in ph3:
                        pass
                for _ in phase3_gen(n):
                    pass

        assert not filler and not slots


_CACHE = {}


def _build():
    if "nc" not in _CACHE:
        nc = bacc.Bacc("TRN2", target_bir_lowering=False, debug=False,
                       num_devices=NCORES)
        with tile.TileContext(nc) as tc:
            _emit(tc)
        nc.compile()
        _CACHE["nc"] = nc
    return _CACHE["nc"]


def _in_maps(x, W_qkv, W_proj):
    in_maps = []
    for c in range(NCORES):
        b, g = divmod(c, 2)
        r0 = g * HD
        w_rows = np.concatenate([
            W_qkv[0 * EMB + r0: 0 * EMB + r0 + HD],
            W_qkv[1 * EMB + r0: 1 * EMB + r0 + HD],
            W_qkv[2 * EMB + r0: 2 * EMB + r0 + HD],
        ], axis=0)                                   # [1152, 768]
        bf = ml_dtypes.bfloat16
        in_maps.append({
            "xT": np.ascontiguousarray(x[b].T.astype(bf)),
            "wT": np.ascontiguousarray(w_rows.T.astype(bf)),
            "wpT": np.ascontiguousarray(W_proj[:, r0:r0 + HD].T.astype(bf)),
        })
    return in_maps


LAST_RESULTS = None


def kernel(x, W_qkv, W_proj, b_proj):
    global LAST_RESULTS
    x = np.ascontiguousarray(np.asarray(x, dtype=np.float32))
    W_qkv = np.asarray(W_qkv, dtype=np.float32)
    W_proj = np.asarray(W_proj, dtype=np.float32)
    b_proj = np.asarray(b_proj, dtype=np.float32)

    nc = _build()
    in_maps = _in_maps(x, W_qkv, W_proj)
    res = run_bass_kernel_spmd(nc, in_maps, core_ids=list(range(NCORES)))
    LAST_RESULTS = res

    out = np.empty((B, N, EMB), dtype=np.float32)
    for b in range(B):
        out[b] = res.results[2 * b]["outp"] + res.results[2 * b + 1]["outp"]
    out += b_proj
    return out
